# revision 16
# baseline (speedup 1.0000x reference)
"""Trainium2 Bass kernel for nn_Attn_74242804679156 (sparse_attention).

Reference computation:
    h = hidden[0]                                  # [B, H]
    energy[b, s] = <h_b, enc[s, b, :]> + <h_b @ affect_matrix, emb[s, b, :]>
    out = softmax(energy, axis=s)[:, None, :]      # [B, 1, S]

Strategy (B=64 sharded 8 ways -> 8 batches/core, data parallel):
  Pure streaming problem (268MB of encoder_outputs read once) - runtime
  == bytes moved. Default variant "v6" streams enc as fp8e4 (8.4MB/core,
  ~16us DMA floor vs ~47us for fp16) and fixes fp8's ~0.85 energy error
  with an exact top-8 patch pass:

  * pass 1 (PE): DoubleRow fp8e4 matmuls (0.5 cyc/row; stationary padded
    to 16 cols - ISA requires M>=16) accumulate energy [8, 2048] in PSUM
    from 16 paired-k-chunk moving blocks [128, 2, 2048]. The affect term
    rides along as 3 fp16 contraction rows per batch (v = h@affect_matrix
    as stationary, emb^T as moving) - no host-side energy math.
  * pass 2: the top-8 energies per row are found with vector.max /
    max_index, and match_replace simultaneously masks them out of the
    tail at -1e30. Their exact fp16 energies are recomputed on-device
    from a gather table (row s*8+b = [enc16, emb16]) fetched via
    indirect DMA, exp'd, added to the softmax denominator, and patch-
    scattered into the output via indirect DMA. Unpatched entries are
    >~6 below the row max (tail mass < 0.2%), so their fp8 error moves
    the output by < 1e-3; overall rel err ~5e-3 (fp16-patch-dominated,
    same as an all-fp16 kernel; gate is 2e-2).
  * overlap: two-stage software pipeline. stageB(r-2) and stageA(r-1)
    are emitted after pass1(r)'s DMA issues so no stream queue ever
    waits on late epilogue compute, and the indirect gather gets a full
    iteration of latency budget. Epilogue DMAs that cross partition
    layouts run on the gpsimd (SWDGE) queue, off the stream queues.

  Probe-verified HW constraints baked in here:
  * tensor_tensor_reduce crashes the exec unit (NRT_EXEC_UNIT_
    UNRECOVERABLE) - the exact-dot uses tensor_tensor mult + reduce_sum.
  * indirect DMA offset APs must be [n, 1] partition-aligned; [8, 16]-
    shaped offset APs crash. Candidate indices are rearranged to
    [64, 1] via a small SWDGE DMA first.
  * DoubleRow needs stationary free width >= 16 (walrus ISA check).

  "full" (fp16 stream, ~47us) and "v3" (fp8e3 + top-16, ~23us) variants
  are kept for A/B testing via ATTN_VARIANT.
"""

import os

import numpy as np

import concourse.bacc as bacc
import concourse.tile as tile
from concourse import mybir
from concourse._compat import with_exitstack
from concourse.bass import IndirectOffsetOnAxis
from concourse.bass_utils import run_bass_kernel_spmd

# Problem shape (hardcoded per contract)
B, S, H, A = 64, 2048, 512, 3
NCORES = 8
BPC = B // NCORES   # 8 batches per core
P = 128             # SBUF partitions
KC = H // P         # 4 k-chunks per batch
NBLK = BPC * KC     # 32 moving blocks per core
MMF = 512           # matmul moving free width (one PSUM bank of fp32)
NK = 16             # patched candidates per row (2 rounds of max8)
PW = 516            # gather row width: enc16(512) + emb16(3) + pad
F32 = mybir.dt.float32
F16 = mybir.dt.float16
F8E3 = mybir.dt.float8e3
I32 = mybir.dt.int32
U32 = mybir.dt.uint32

DEFAULT_VARIANT = os.environ.get("ATTN_VARIANT", "v6")

# Last BassKernelResults (for test harness to read exec_time_ns)
LAST_RESULTS = None


@with_exitstack
def emit_kernel(ctx, tc, out_ap, x_ap, stat_ap, aff_ap, reps=1):
    """fp16 "full" variant: stationary one-hot-column h blocks, fp16 enc
    stream, host-precomputed aff bias, softmax epilogue."""
    nc = tc.nc
    xv = x_ap.rearrange("(n p) s -> n p s", p=P)          # [32, 128, 2048]
    statv = stat_ap.rearrange("p (n j) -> p n j", j=BPC)  # [128, 32, 8]

    singles = ctx.enter_context(tc.tile_pool(name="singles", bufs=1))
    bpool = ctx.enter_context(tc.tile_pool(name="blocks", bufs=8))
    smpool = ctx.enter_context(tc.tile_pool(name="smx", bufs=2))
    epool = ctx.enter_context(tc.tile_pool(name="es", bufs=4))
    ppool = ctx.enter_context(tc.tile_pool(name="psums", bufs=2, space="PSUM"))

    statt = singles.tile([P, NBLK, BPC], F16)
    nc.gpsimd.dma_start(out=statt[:, :, :], in_=statv)
    afft = singles.tile([BPC, S], F32)
    nc.gpsimd.dma_start(out=afft[:, :], in_=aff_ap)

    nmm = S // MMF
    for _ in range(reps):
        energy = ppool.tile([BPC, S], F32)
        for i in range(NBLK):
            blk = bpool.tile([P, S], F16)
            q = nc.sync if i % 2 == 0 else nc.scalar
            q.dma_start(out=blk[:, :], in_=xv[i])
            first = i == 0
            last = i == NBLK - 1
            for sc in range(nmm):
                nc.tensor.matmul(
                    energy[:, sc * MMF : (sc + 1) * MMF],
                    statt[:, i, :],
                    blk[:, sc * MMF : (sc + 1) * MMF],
                    start=first,
                    stop=last,
                )

        eng = smpool.tile([BPC, S], F32)
        nc.vector.tensor_tensor(
            eng[:, :], energy[:, :], afft[:, :], mybir.AluOpType.add
        )
        negmax1 = epool.tile([BPC, 1], F32)
        nc.vector.reduce_max(
            negmax1[:, :], eng[:, : S // 2], axis=mybir.AxisListType.X,
            negate=True,
        )
        negmax = epool.tile([BPC, 1], F32)
        nc.vector.reduce_max(
            negmax[:, :], eng[:, S // 2 :], axis=mybir.AxisListType.X,
            negate=True,
        )
        nc.vector.tensor_tensor(
            negmax[:, :], negmax[:, :], negmax1[:, :], mybir.AluOpType.min
        )
        expT = smpool.tile([BPC, S], F32)
        sums = epool.tile([BPC, 1], F32)
        nc.scalar.activation(
            expT[:, :],
            eng[:, :],
            mybir.ActivationFunctionType.Exp,
            bias=negmax[:, :],
            scale=1.0,
            accum_out=sums[:, :],
        )
        rsum = epool.tile([BPC, 1], F32)
        nc.vector.reciprocal(rsum[:, :], sums[:, :])
        outT = smpool.tile([BPC, S], F32)
        nc.scalar.activation(
            outT[:, :],
            expT[:, :],
            mybir.ActivationFunctionType.Copy,
            bias=0.0,
            scale=rsum[:, :],
        )
        nc.sync.dma_start(out=out_ap, in_=outT[:, :])


@with_exitstack
def emit_kernel_v3(ctx, tc, out_ap, x_ap, stat_ap, embt_ap, vstat_ap, p2_ap,
                   hsel_ap, bc8_ap, bc2048_ap, reps=1, stage="all",
                   pipelined=True):
    """fp8e3 two-pass variant. stage: 'p1' = pass-1 only (timing floor),
    'nog' = extraction but no gather/patch (plain softmax of fp8 energies
    with top-16 zeroed - wrong output, DVE-chain timing), 'nos' = gather
    but no scatter, 'all' = full."""
    nc = tc.nc
    xv = x_ap.rearrange("(n p) s -> n p s", p=P)          # [32, 128, 2048] f8
    statv = stat_ap.rearrange("p (n j) -> p n j", j=BPC)  # [128, 32, 8] f8
    outv = out_ap.rearrange("(b s) o -> b (s o)", b=BPC)  # [8, 2048]

    singles = ctx.enter_context(tc.tile_pool(name="singles", bufs=1))
    bpool = ctx.enter_context(tc.tile_pool(name="blocks", bufs=8))
    empool = ctx.enter_context(tc.tile_pool(name="embts", bufs=2))
    spool = ctx.enter_context(tc.tile_pool(name="sm", bufs=2))
    tpool = ctx.enter_context(tc.tile_pool(name="tiny", bufs=3))
    gpool = ctx.enter_context(tc.tile_pool(name="gath", bufs=2))
    ppool = ctx.enter_context(tc.tile_pool(name="psums", bufs=2, space="PSUM"))

    # one-time loads on the gpsimd (SWDGE) queue
    statt = singles.tile([P, NBLK, BPC], F8E3)
    nc.gpsimd.dma_start(out=statt[:, :, :], in_=statv)
    vstatt = singles.tile([BPC * A, BPC], F16)
    nc.gpsimd.dma_start(out=vstatt[:, :], in_=vstat_ap)
    hselt = singles.tile([P, PW], F16)
    nc.gpsimd.dma_start(out=hselt[:, :], in_=hsel_ap)
    bc8t = singles.tile([P, 1], F32)
    nc.gpsimd.dma_start(out=bc8t[:, :], in_=bc8_ap)
    bc2048t = singles.tile([P, 1], F32)
    nc.gpsimd.dma_start(out=bc2048t[:, :], in_=bc2048_ap)

    nmm = S // MMF

    if stage == "dma":
        outT0 = singles.tile([BPC, S], F32)
        nc.vector.memset(outT0[:, :], 0.0)
        for _ in range(reps):
            for i in range(NBLK):
                blk = bpool.tile([P, S], F8E3)
                q = nc.sync if i % 2 == 0 else nc.scalar
                q.dma_start(out=blk[:, :], in_=xv[i])
                ec = tpool.tile([P, 1], F8E3)
                nc.vector.tensor_copy(ec[:, :], blk[:, 0:1])
            embt = empool.tile([BPC * A, S], F16)
            nc.scalar.dma_start(out=embt[:, :], in_=embt_ap)
            ec2 = tpool.tile([BPC * A, 1], F16)
            nc.vector.tensor_copy(ec2[:, :], embt[:, 0:1])
        nc.gpsimd.dma_start(out=outv, in_=outT0[:, :])
        return

    def pass1():
        # ---- pass 1: energy [8, 2048] accumulated in PSUM ----
        energy = ppool.tile([BPC, S], F32)
        for i in range(NBLK):
            blk = bpool.tile([P, S], F8E3)
            q = nc.sync if i % 2 == 0 else nc.scalar
            q.dma_start(out=blk[:, :], in_=xv[i])
            for sc in range(nmm):
                nc.tensor.matmul(
                    energy[:, sc * MMF : (sc + 1) * MMF],
                    statt[:, i, :],
                    blk[:, sc * MMF : (sc + 1) * MMF],
                    start=(i == 0),
                    stop=False,
                )
        # affect term: 3 fp16 contraction rows per batch
        embt = empool.tile([BPC * A, S], F16)
        nc.scalar.dma_start(out=embt[:, :], in_=embt_ap)
        for sc in range(nmm):
            nc.tensor.matmul(
                energy[:, sc * MMF : (sc + 1) * MMF],
                vstatt[:, :],
                embt[:, sc * MMF : (sc + 1) * MMF],
                start=False,
                stop=True,
            )
        return energy

    def stageA(energy):
        """Extraction + gather launch. The gather's latency is absorbed by
        running stageB one iteration later."""
        eng = spool.tile([BPC, S], F32)
        nc.vector.tensor_copy(eng[:, :], energy[:, :])  # frees PSUM buf

        if stage == "p1":
            e0 = tpool.tile([BPC, 1], F32)
            nc.vector.tensor_copy(e0[:, :], eng[:, 0:1])
            nc.sync.dma_start(out=outv[:, 0:1], in_=e0[:, :])
            return None

        # top-16 per row: values+indices, masked out of the tail in place
        m1 = tpool.tile([BPC, 8], F32)
        nc.vector.max(m1[:, :], eng[:, :])
        iall = tpool.tile([BPC, NK], U32)
        nc.vector.max_index(iall[:, 0:8], m1[:, :], eng[:, :])
        eng2 = spool.tile([BPC, S], F32)
        nc.vector.match_replace(eng2[:, :], m1[:, :], eng[:, :], -1e30)
        m2 = tpool.tile([BPC, 8], F32)
        nc.vector.max(m2[:, :], eng2[:, :])
        nc.vector.max_index(iall[:, 8:16], m2[:, :], eng2[:, :])
        eng3 = spool.tile([BPC, S], F32)
        nc.vector.match_replace(eng3[:, :], m2[:, :], eng2[:, :], -1e30)

        negmax = tpool.tile([BPC, 1], F32)
        nc.vector.tensor_scalar_mul(negmax[:, :], m1[:, 0:1], -1.0)
        if32 = tpool.tile([BPC, NK], F32)
        nc.vector.tensor_copy(if32[:, :], iall[:, :])
        # rearrange candidates [8, 16] -> [128, 1] (partition-major)
        cidx = gpool.tile([P, 1], F32)
        nc.gpsimd.dma_start(out=cidx[:, :], in_=if32[:, :])

        G = None
        if stage not in ("nog",):
            # gather exact fp16 rows: p2 row = s*8 + b
            crowf = gpool.tile([P, 1], F32)
            nc.vector.tensor_scalar_mul(crowf[:, :], cidx[:, :], 8.0)
            nc.vector.tensor_tensor(
                crowf[:, :], crowf[:, :], bc8t[:, :], mybir.AluOpType.add
            )
            crow = gpool.tile([P, 1], I32)
            nc.vector.tensor_copy(crow[:, :], crowf[:, :])
            G = gpool.tile([P, PW], F16)
            nc.gpsimd.indirect_dma_start(
                out=G[:, :],
                out_offset=None,
                in_=p2_ap,
                in_offset=IndirectOffsetOnAxis(ap=crow[:, 0:1], axis=0),
            )
        return (eng3, negmax, cidx, G)

    def stageB(st):
        eng3, negmax, cidx, G = st
        if stage not in ("nog",):
            # exact energy per candidate
            prod = gpool.tile([P, PW], F32)
            nc.vector.tensor_tensor(
                prod[:, :], G[:, :], hselt[:, :], mybir.AluOpType.mult
            )
            ee = gpool.tile([P, 1], F32)
            nc.vector.reduce_sum(
                ee[:, 0:1], prod[:, :], axis=mybir.AxisListType.X
            )
        if stage not in ("nog", "gonly"):
            # back to [8, 16] layout for per-row reduction
            eeb = tpool.tile([BPC, NK], F32)
            nc.gpsimd.dma_start(out=eeb[:, :], in_=ee[:, :])
            expfix = tpool.tile([BPC, NK], F32)
            nc.scalar.activation(
                expfix[:, :], eeb[:, :],
                mybir.ActivationFunctionType.Exp,
                bias=negmax[:, :], scale=1.0,
            )
            psumf = tpool.tile([BPC, 1], F32)
            nc.vector.reduce_sum(
                psumf[:, 0:1], expfix[:, :], axis=mybir.AxisListType.X
            )

        # tail softmax (top-16 already -1e30 in eng3)
        exps = spool.tile([BPC, S], F32)
        tsum = tpool.tile([BPC, 1], F32)
        nc.scalar.activation(
            exps[:, :],
            eng3[:, :],
            mybir.ActivationFunctionType.Exp,
            bias=negmax[:, :],
            scale=1.0,
            accum_out=tsum[:, :],
        )
        zt = tpool.tile([BPC, 1], F32)
        if stage == "gonly":
            # consume ee so the gather isn't dead code; ee*0 keeps values
            nc.vector.scalar_tensor_tensor(
                out=zt[:, :], in0=ee[0:BPC, 0:1], scalar=0.0,
                in1=tsum[:, :],
                op0=mybir.AluOpType.mult, op1=mybir.AluOpType.add,
            )
        elif stage != "nog":
            nc.vector.tensor_tensor(
                zt[:, :], tsum[:, :], psumf[:, :], mybir.AluOpType.add
            )
        else:
            nc.vector.tensor_copy(zt[:, :], tsum[:, :])
        rsum = tpool.tile([BPC, 1], F32)
        nc.vector.reciprocal(rsum[:, :], zt[:, :])
        outT = spool.tile([BPC, S], F32)
        nc.vector.tensor_scalar_mul(outT[:, :], exps[:, :], rsum[:, 0:1])

        # base write then sparse patches, both on gpsimd queue (ordered)
        nc.gpsimd.dma_start(out=outv, in_=outT[:, :])
        if stage == "all":
            pv8 = tpool.tile([BPC, NK], F32)
            nc.vector.tensor_scalar_mul(pv8[:, :], expfix[:, :], rsum[:, 0:1])
            pv = gpool.tile([P, 1], F32)
            nc.gpsimd.dma_start(out=pv[:, :], in_=pv8[:, :])
            offf = gpool.tile([P, 1], F32)
            nc.vector.tensor_tensor(
                offf[:, :], cidx[:, :], bc2048t[:, :], mybir.AluOpType.add
            )
            offi = gpool.tile([P, 1], I32)
            nc.vector.tensor_copy(offi[:, :], offf[:, :])
            nc.gpsimd.indirect_dma_start(
                out=out_ap,
                out_offset=IndirectOffsetOnAxis(ap=offi[:, 0:1], axis=0),
                in_=pv[:, :],
                in_offset=None,
            )

    # two-stage software pipeline: stageB(r-2) then stageA(r-1) are emitted
    # after pass1(r), so (a) every queue sees the next iteration's DMA
    # issues before the late-ready epilogue compute, and (b) the indirect
    # gather launched in stageA(r) has a full iteration of latency budget
    # before stageB(r) consumes it - without this the base-output write
    # (which needs Z = tail + patch sum) serializes on the gather roundtrip.
    if pipelined:
        prevE = None
        prevS = None
        for _ in range(reps):
            cur = pass1()
            if prevS is not None:
                stageB(prevS)
                prevS = None
            if prevE is not None:
                prevS = stageA(prevE)
            prevE = cur
        if prevS is not None:
            stageB(prevS)
        if prevE is not None:
            st = stageA(prevE)
            if st is not None:
                stageB(st)
    else:
        for _ in range(reps):
            st = stageA(pass1())
            if st is not None:
                stageB(st)


NK6 = 8              # v6: one max round
NC6 = BPC * NK6      # 64 gather candidates
NBLK6 = NBLK // 2    # 16 paired moving blocks
F8E4 = mybir.dt.float8e4


@with_exitstack
def emit_kernel_v6(ctx, tc, out_ap, x_ap, stat_ap, embt_ap, vstat_ap, p2_ap,
                   hsel_ap, bc8_ap, bc2048_ap, reps=1, stage="all"):
    """DoubleRow fp8e4 pass-1 (0.5 cyc/row) + top-8 exact-patch pass-2.
    Depth-2 software pipeline as in v3."""
    nc = tc.nc
    # block j = b*2 + kp: [128, 2, 2048]; row p, ktile t = enc[s, b, (2kp+t)*128+p]
    xv = x_ap.rearrange("(n p) (t s) -> n p t s", p=P, t=2)   # [16,128,2,2048]
    # stationary free dim padded to 16 (DoubleRow ISA requires M >= 16)
    statv = stat_ap.rearrange("p (n t j) -> p n t j", t=2, j=2 * BPC)
    outv = out_ap.rearrange("(b s) o -> b (s o)", b=BPC)      # [8, 2048]

    singles = ctx.enter_context(tc.tile_pool(name="singles", bufs=1))
    bpool = ctx.enter_context(tc.tile_pool(name="blocks", bufs=6))
    empool = ctx.enter_context(tc.tile_pool(name="embts", bufs=2))
    spool = ctx.enter_context(tc.tile_pool(name="sm", bufs=2))
    tpool = ctx.enter_context(tc.tile_pool(name="tiny", bufs=3))
    gpool = ctx.enter_context(tc.tile_pool(name="gath", bufs=2))
    ppool = ctx.enter_context(tc.tile_pool(name="psums", bufs=2, space="PSUM"))

    statt = singles.tile([P, NBLK6, 2, 2 * BPC], F8E4)
    nc.gpsimd.dma_start(out=statt[:, :, :, :], in_=statv)
    vstatt = singles.tile([BPC * A, 2 * BPC], F16)
    nc.gpsimd.dma_start(out=vstatt[:, :], in_=vstat_ap)
    hselt = singles.tile([NC6, PW], F16)
    nc.gpsimd.dma_start(out=hselt[:, :], in_=hsel_ap)
    bc8t = singles.tile([NC6, 1], F32)
    nc.gpsimd.dma_start(out=bc8t[:, :], in_=bc8_ap)
    bc2048t = singles.tile([NC6, 1], F32)
    nc.gpsimd.dma_start(out=bc2048t[:, :], in_=bc2048_ap)

    nmm = S // MMF

    def pass1():
        energy = ppool.tile([2 * BPC, S], F32)
        for j in range(NBLK6):
            blk = bpool.tile([P, 2, S], F8E4)
            q = nc.sync if j % 2 == 0 else nc.scalar
            q.dma_start(out=blk[:, :, :], in_=xv[j])
            for sc in range(nmm):
                nc.tensor.matmul(
                    energy[:, sc * MMF : (sc + 1) * MMF],
                    statt[:, j, :, :],
                    blk[:, :, sc * MMF : (sc + 1) * MMF],
                    start=(j == 0),
                    stop=False,
                    perf_mode=mybir.MatmulPerfMode.DoubleRow,
                )
        embt = empool.tile([BPC * A, S], F16)
        nc.scalar.dma_start(out=embt[:, :], in_=embt_ap)
        for sc in range(nmm):
            nc.tensor.matmul(
                energy[:, sc * MMF : (sc + 1) * MMF],
                vstatt[:, :],
                embt[:, sc * MMF : (sc + 1) * MMF],
                start=False,
                stop=True,
            )
        return energy

    def stageA(energy):
        eng = spool.tile([BPC, S], F32)
        nc.vector.tensor_copy(eng[:, :], energy[0:BPC, :])  # frees PSUM buf

        if stage == "p1":
            e0 = tpool.tile([BPC, 1], F32)
            nc.vector.tensor_copy(e0[:, :], eng[:, 0:1])
            nc.sync.dma_start(out=outv[:, 0:1], in_=e0[:, :])
            return None

        m1 = tpool.tile([BPC, 8], F32)
        nc.vector.max(m1[:, :], eng[:, :])
        i8 = tpool.tile([BPC, NK6], U32)
        nc.vector.max_index(i8[:, :], m1[:, :], eng[:, :])
        eng3 = spool.tile([BPC, S], F32)
        nc.vector.match_replace(eng3[:, :], m1[:, :], eng[:, :], -1e30)

        negmax = tpool.tile([BPC, 1], F32)
        nc.vector.tensor_scalar_mul(negmax[:, :], m1[:, 0:1], -1.0)
        if32 = tpool.tile([BPC, NK6], F32)
        nc.vector.tensor_copy(if32[:, :], i8[:, :])
        cidx = gpool.tile([NC6, 1], F32)
        nc.gpsimd.dma_start(out=cidx[:, :], in_=if32[:, :])

        crowf = gpool.tile([NC6, 1], F32)
        nc.vector.tensor_scalar(
            out=crowf[:, :], in0=cidx[:, :], scalar1=8.0, scalar2=bc8t[:, 0:1],
            op0=mybir.AluOpType.mult, op1=mybir.AluOpType.add,
        )
        crow = gpool.tile([NC6, 1], I32)
        nc.vector.tensor_copy(crow[:, :], crowf[:, :])
        G = gpool.tile([NC6, PW], F16)
        nc.gpsimd.indirect_dma_start(
            out=G[:, :],
            out_offset=None,
            in_=p2_ap,
            in_offset=IndirectOffsetOnAxis(ap=crow[:, 0:1], axis=0),
        )
        return (eng3, negmax, cidx, G)

    def stageB(st):
        eng3, negmax, cidx, G = st
        prod = gpool.tile([NC6, PW], F32)
        nc.vector.tensor_tensor(
            prod[:, :], G[:, :], hselt[:, :], mybir.AluOpType.mult
        )
        ee = gpool.tile([NC6, 1], F32)
        nc.vector.reduce_sum(ee[:, 0:1], prod[:, :], axis=mybir.AxisListType.X)
        eeb = tpool.tile([BPC, NK6], F32)
        nc.gpsimd.dma_start(out=eeb[:, :], in_=ee[:, :])
        expfix = tpool.tile([BPC, NK6], F32)
        nc.scalar.activation(
            expfix[:, :], eeb[:, :],
            mybir.ActivationFunctionType.Exp,
            bias=negmax[:, :], scale=1.0,
        )
        psumf = tpool.tile([BPC, 1], F32)
        nc.vector.reduce_sum(
            psumf[:, 0:1], expfix[:, :], axis=mybir.AxisListType.X
        )

        exps = spool.tile([BPC, S], F32)
        tsum = tpool.tile([BPC, 1], F32)
        nc.scalar.activation(
            exps[:, :],
            eng3[:, :],
            mybir.ActivationFunctionType.Exp,
            bias=negmax[:, :],
            scale=1.0,
            accum_out=tsum[:, :],
        )
        zt = tpool.tile([BPC, 1], F32)
        nc.vector.tensor_tensor(
            zt[:, :], tsum[:, :], psumf[:, :], mybir.AluOpType.add
        )
        rsum = tpool.tile([BPC, 1], F32)
        nc.vector.reciprocal(rsum[:, :], zt[:, :])
        outT = spool.tile([BPC, S], F32)
        nc.vector.tensor_scalar_mul(outT[:, :], exps[:, :], rsum[:, 0:1])

        nc.gpsimd.dma_start(out=outv, in_=outT[:, :])
        pv8 = tpool.tile([BPC, NK6], F32)
        nc.vector.tensor_scalar_mul(pv8[:, :], expfix[:, :], rsum[:, 0:1])
        pv = gpool.tile([NC6, 1], F32)
        nc.gpsimd.dma_start(out=pv[:, :], in_=pv8[:, :])
        offf = gpool.tile([NC6, 1], F32)
        nc.vector.tensor_scalar_add(offf[:, :], cidx[:, :], bc2048t[:, 0:1])
        offi = gpool.tile([NC6, 1], I32)
        nc.vector.tensor_copy(offi[:, :], offf[:, :])
        nc.gpsimd.indirect_dma_start(
            out=out_ap,
            out_offset=IndirectOffsetOnAxis(ap=offi[:, 0:1], axis=0),
            in_=pv[:, :],
            in_offset=None,
        )

    prevE = None
    prevS = None
    for _ in range(reps):
        cur = pass1()
        if prevS is not None:
            stageB(prevS)
            prevS = None
        if prevE is not None:
            prevS = stageA(prevE)
        prevE = cur
    if prevS is not None:
        stageB(prevS)
    if prevE is not None:
        st = stageA(prevE)
        if st is not None:
            stageB(st)


def make_in_maps_v6(hidden, encoder_outputs, embedding, affect_matrix):
    import ml_dtypes

    f8 = np.dtype(ml_dtypes.float8_e4m3)
    hidden = np.asarray(hidden, dtype=np.float32)
    enc = np.asarray(encoder_outputs, dtype=np.float32)
    emb = np.asarray(embedding, dtype=np.float32)
    am = np.asarray(affect_matrix, dtype=np.float32)

    h = hidden[0]
    v32 = h @ am
    h8 = h.astype(f8)
    enc8 = enc.astype(f8)
    h16 = h.astype(np.float16)
    v16 = v32.astype(np.float16)
    enc16 = enc.astype(np.float16)
    emb16 = emb.astype(np.float16)

    cc = np.arange(NC6)
    bc8 = (cc // NK6).astype(np.float32)[:, None]
    bc2048 = (cc // NK6 * S).astype(np.float32)[:, None]

    in_maps = []
    for c in range(NCORES):
        lo, hi = c * BPC, (c + 1) * BPC
        # [S, 8, 512] -> [8b, 2kp, 2t, 128p, 2048s] -> rows (b,kp,p), cols (t,s)
        ec = np.transpose(enc8[:, lo:hi, :], (1, 2, 0)).reshape(
            BPC, KC // 2, 2, P, S
        )
        xp = np.ascontiguousarray(
            np.transpose(ec, (0, 1, 3, 2, 4))
        ).reshape(NBLK6 * P, 2 * S)
        hh8 = h8[lo:hi].reshape(BPC, KC // 2, 2, P)
        stat = np.zeros((P, NBLK6, 2, 2 * BPC), f8)
        for b in range(BPC):
            for kp in range(KC // 2):
                for t in range(2):
                    stat[:, b * (KC // 2) + kp, t, b] = hh8[b, kp, t]
        embt = np.ascontiguousarray(
            np.transpose(emb16[:, lo:hi, :], (1, 2, 0))
        ).reshape(BPC * A, S)
        vstat = np.zeros((BPC * A, 2 * BPC), np.float16)
        for b in range(BPC):
            vstat[b * A : (b + 1) * A, b] = v16[lo + b]
        p2 = np.zeros((S * BPC, PW), np.float16)
        p2[:, :H] = enc16[:, lo:hi, :].reshape(S * BPC, H)
        p2[:, H : H + A] = emb16[:, lo:hi, :].reshape(S * BPC, A)
        hsel = np.zeros((NC6, PW), np.float16)
        hsel[:, :H] = h16[lo + cc // NK6]
        hsel[:, H : H + A] = v16[lo + cc // NK6]
        in_maps.append(
            {
                "x": xp,
                "stat": stat.reshape(P, NBLK6 * 2 * 2 * BPC),
                "embt": embt,
                "vstat": vstat,
                "p2": p2,
                "hsel": hsel,
                "bc8": bc8,
                "bc2048": bc2048,
            }
        )
    return in_maps


_NC_CACHE = {}


def build_nc(reps=1, variant="v3"):
    key = (reps, variant)
    if key in _NC_CACHE:
        return _NC_CACHE[key]
    nc = bacc.Bacc(
        "TRN2",
        target_bir_lowering=False,
        debug=False,
        enable_asserts=False,
        num_devices=NCORES,
    )
    if variant.startswith("v6"):
        x = nc.dram_tensor(
            "x", [NBLK6 * P, 2 * S], F8E4, kind="ExternalInput"
        ).ap()
        stat = nc.dram_tensor(
            "stat", [P, NBLK6 * 2 * 2 * BPC], F8E4, kind="ExternalInput"
        ).ap()
        embt = nc.dram_tensor(
            "embt", [BPC * A, S], F16, kind="ExternalInput"
        ).ap()
        vstat = nc.dram_tensor(
            "vstat", [BPC * A, 2 * BPC], F16, kind="ExternalInput"
        ).ap()
        p2 = nc.dram_tensor(
            "p2", [S * BPC, PW], F16, kind="ExternalInput"
        ).ap()
        hsel = nc.dram_tensor(
            "hsel", [NC6, PW], F16, kind="ExternalInput"
        ).ap()
        bc8 = nc.dram_tensor("bc8", [NC6, 1], F32, kind="ExternalInput").ap()
        bc2048 = nc.dram_tensor(
            "bc2048", [NC6, 1], F32, kind="ExternalInput"
        ).ap()
        out = nc.dram_tensor(
            "out", [BPC * S, 1], F32, kind="ExternalOutput"
        ).ap()
        stage = {"v6p1": "p1"}.get(variant, "all")
        with tile.TileContext(nc) as tc:
            emit_kernel_v6(
                tc, out, x, stat, embt, vstat, p2, hsel, bc8, bc2048,
                reps=reps, stage=stage,
            )
    elif variant.startswith("v3"):
        x = nc.dram_tensor("x", [NBLK * P, S], F8E3, kind="ExternalInput").ap()
        stat = nc.dram_tensor(
            "stat", [P, NBLK * BPC], F8E3, kind="ExternalInput"
        ).ap()
        embt = nc.dram_tensor(
            "embt", [BPC * A, S], F16, kind="ExternalInput"
        ).ap()
        vstat = nc.dram_tensor(
            "vstat", [BPC * A, BPC], F16, kind="ExternalInput"
        ).ap()
        p2 = nc.dram_tensor(
            "p2", [S * BPC, PW], F16, kind="ExternalInput"
        ).ap()
        hsel = nc.dram_tensor("hsel", [P, PW], F16, kind="ExternalInput").ap()
        bc8 = nc.dram_tensor("bc8", [P, 1], F32, kind="ExternalInput").ap()
        bc2048 = nc.dram_tensor(
            "bc2048", [P, 1], F32, kind="ExternalInput"
        ).ap()
        out = nc.dram_tensor(
            "out", [BPC * S, 1], F32, kind="ExternalOutput"
        ).ap()
        stage = {"v3p1": "p1", "v3nog": "nog", "v3nos": "nos",
                 "v3gonly": "gonly", "v3dma": "dma",
                 "v3s": "all"}.get(variant, "all")
        with tile.TileContext(nc) as tc:
            emit_kernel_v3(
                tc, out, x, stat, embt, vstat, p2, hsel, bc8, bc2048,
                reps=reps, stage=stage, pipelined=(variant != "v3s"),
            )
    else:
        x = nc.dram_tensor("x", [NBLK * P, S], F16, kind="ExternalInput").ap()
        stat = nc.dram_tensor(
            "stat", [P, NBLK * BPC], F16, kind="ExternalInput"
        ).ap()
        aff = nc.dram_tensor("aff", [BPC, S], F32, kind="ExternalInput").ap()
        out = nc.dram_tensor("out", [BPC, S], F32, kind="ExternalOutput").ap()
        with tile.TileContext(nc) as tc:
            emit_kernel(tc, out, x, stat, aff, reps=reps)
    nc.compile()
    _NC_CACHE[key] = nc
    return nc


def make_in_maps(hidden, encoder_outputs, embedding, affect_matrix):
    hidden = np.asarray(hidden, dtype=np.float32)
    enc = np.asarray(encoder_outputs, dtype=np.float32)
    emb = np.asarray(embedding, dtype=np.float32)
    am = np.asarray(affect_matrix, dtype=np.float32)

    h = hidden[0]                                   # [B, H]
    v = h @ am                                      # [B, A]
    aff = np.einsum("ba,sba->sb", v, emb).astype(np.float32)  # [S, B]
    h16 = h.astype(np.float16)
    enc16 = enc.astype(np.float16)                  # [S, B, H]

    in_maps = []
    for c in range(NCORES):
        lo, hi = c * BPC, (c + 1) * BPC
        xp = np.ascontiguousarray(
            np.transpose(enc16[:, lo:hi, :], (1, 2, 0))
        ).reshape(BPC * H, S)
        hh = h16[lo:hi].reshape(BPC, KC, P)
        stat = np.zeros((P, NBLK, BPC), np.float16)
        for b in range(BPC):
            for kc in range(KC):
                stat[:, b * KC + kc, b] = hh[b, kc]
        in_maps.append(
            {
                "x": xp,
                "stat": stat.reshape(P, NBLK * BPC),
                "aff": np.ascontiguousarray(aff[:, lo:hi].T),
            }
        )
    return in_maps


def make_in_maps_v3(hidden, encoder_outputs, embedding, affect_matrix):
    import ml_dtypes

    f8 = np.dtype(ml_dtypes.float8_e3m4)
    hidden = np.asarray(hidden, dtype=np.float32)
    enc = np.asarray(encoder_outputs, dtype=np.float32)
    emb = np.asarray(embedding, dtype=np.float32)
    am = np.asarray(affect_matrix, dtype=np.float32)

    h = hidden[0]                                   # [B, H]
    v32 = h @ am                                    # [B, A]
    h8 = h.astype(f8)
    enc8 = enc.astype(f8)
    h16 = h.astype(np.float16)
    v16 = v32.astype(np.float16)
    enc16 = enc.astype(np.float16)
    emb16 = emb.astype(np.float16)

    cc = np.arange(P)
    bc8 = (cc // NK).astype(np.float32)[:, None]
    bc2048 = (cc // NK * S).astype(np.float32)[:, None]

    in_maps = []
    for c in range(NCORES):
        lo, hi = c * BPC, (c + 1) * BPC
        xp = np.ascontiguousarray(
            np.transpose(enc8[:, lo:hi, :], (1, 2, 0))
        ).reshape(NBLK * P, S)
        hh8 = h8[lo:hi].reshape(BPC, KC, P)
        stat = np.zeros((P, NBLK, BPC), f8)
        for b in range(BPC):
            for kc in range(KC):
                stat[:, b * KC + kc, b] = hh8[b, kc]
        # affect rows: embt[b*A+a, s] = emb[s, lo+b, a]
        embt = np.ascontiguousarray(
            np.transpose(emb16[:, lo:hi, :], (1, 2, 0))
        ).reshape(BPC * A, S)
        vstat = np.zeros((BPC * A, BPC), np.float16)
        for b in range(BPC):
            vstat[b * A : (b + 1) * A, b] = v16[lo + b]
        # gather table rows r = s*8 + b: [enc16(512), emb16(3), pad]
        p2 = np.zeros((S * BPC, PW), np.float16)
        p2[:, :H] = enc16[:, lo:hi, :].reshape(S * BPC, H)
        p2[:, H : H + A] = emb16[:, lo:hi, :].reshape(S * BPC, A)
        hsel = np.zeros((P, PW), np.float16)
        hsel[:, :H] = h16[lo + cc // NK]
        hsel[:, H : H + A] = v16[lo + cc // NK]
        in_maps.append(
            {
                "x": xp,
                "stat": stat.reshape(P, NBLK * BPC),
                "embt": embt,
                "vstat": vstat,
                "p2": p2,
                "hsel": hsel,
                "bc8": bc8,
                "bc2048": bc2048,
            }
        )
    return in_maps


def kernel(hidden, encoder_outputs, embedding, affect_matrix):
    global LAST_RESULTS
    variant = DEFAULT_VARIANT
    nc = build_nc(variant=variant)
    if variant.startswith("v6"):
        in_maps = make_in_maps_v6(
            hidden, encoder_outputs, embedding, affect_matrix
        )
    elif variant.startswith("v3"):
        in_maps = make_in_maps_v3(
            hidden, encoder_outputs, embedding, affect_matrix
        )
    else:
        in_maps = make_in_maps(
            hidden, encoder_outputs, embedding, affect_matrix
        )
    last_exc = None
    for attempt in range(3):
        try:
            res = run_bass_kernel_spmd(
                nc,
                in_maps,
                core_ids=list(range(NCORES)),
                trace=bool(int(os.environ.get("ATTN_TRACE", "0"))),
            )
            break
        except Exception as e:  # transient wedged-device errors recover on retry
            last_exc = e
            if attempt == 2:
                raise
            import time as _time

            _time.sleep(5.0)
    LAST_RESULTS = res
    outs = [r["out"].reshape(BPC, S) for r in res.results]
    full = np.concatenate(outs, axis=0)             # [B, S]
    return full[:, None, :].astype(np.float32)      # [B, 1, S]


# revision 19
# speedup vs baseline: 1.0259x; 1.0259x over previous
"""Trainium2 Bass kernel for nn_Attn_74242804679156 (sparse_attention).

Reference computation:
    h = hidden[0]                                  # [B, H]
    energy[b, s] = <h_b, enc[s, b, :]> + <h_b @ affect_matrix, emb[s, b, :]>
    out = softmax(energy, axis=s)[:, None, :]      # [B, 1, S]

Strategy (B=64 sharded 8 ways -> 8 batches/core, data parallel):
  Pure streaming problem (268MB of encoder_outputs read once) - runtime
  == bytes moved. Default variant "v6" streams enc as fp8e4 (8.4MB/core,
  ~16us DMA floor vs ~47us for fp16) and fixes fp8's ~0.85 energy error
  with an exact top-8 patch pass:

  * pass 1 (PE): DoubleRow fp8e4 matmuls (0.5 cyc/row; stationary padded
    to 16 cols - ISA requires M>=16) accumulate energy [8, 2048] in PSUM
    from 16 paired-k-chunk moving blocks [128, 2, 2048]. The affect term
    rides along as 3 fp16 contraction rows per batch (v = h@affect_matrix
    as stationary, emb^T as moving) - no host-side energy math.
  * pass 2: the top-8 energies per row are found with vector.max /
    max_index, and match_replace simultaneously masks them out of the
    tail at -1e30. Their exact fp16 energies are recomputed on-device
    from a gather table (row s*8+b = [enc16, emb16]) fetched via
    indirect DMA, exp'd, added to the softmax denominator, and patch-
    scattered into the output via indirect DMA. Unpatched entries are
    >~6 below the row max (tail mass < 0.2%), so their fp8 error moves
    the output by < 1e-3; overall rel err ~5e-3 (fp16-patch-dominated,
    same as an all-fp16 kernel; gate is 2e-2).
  * overlap: two-stage software pipeline. stageB(r-2) and stageA(r-1)
    are emitted after pass1(r)'s DMA issues so no stream queue ever
    waits on late epilogue compute, and the indirect gather gets a full
    iteration of latency budget. Epilogue DMAs that cross partition
    layouts run on the gpsimd (SWDGE) queue, off the stream queues.
  * the block stream is split across THREE DMA queues (sync/scalar/
    gpsimd, default 6/5/5 blocks) - per-queue issue rate, not aggregate
    HBM bandwidth, limits a 2-queue stream (~2.5us win, A/B-measured).

  Probe-verified HW constraints baked in here:
  * tensor_tensor_reduce crashes the exec unit (NRT_EXEC_UNIT_
    UNRECOVERABLE) - the exact-dot uses tensor_tensor mult + reduce_sum.
  * indirect DMA offset APs must be [n, 1] partition-aligned; [8, 16]-
    shaped offset APs crash. Candidate indices are rearranged to
    [64, 1] via a small SWDGE DMA first.
  * DoubleRow needs stationary free width >= 16 (walrus ISA check).

  "full" (fp16 stream, ~47us) and "v3" (fp8e3 + top-16, ~23us) variants
  are kept for A/B testing via ATTN_VARIANT.
"""

import os

import numpy as np

import concourse.bacc as bacc
import concourse.tile as tile
from concourse import mybir
from concourse._compat import with_exitstack
from concourse.bass import IndirectOffsetOnAxis
from concourse.bass_utils import run_bass_kernel_spmd

# Problem shape (hardcoded per contract)
B, S, H, A = 64, 2048, 512, 3
NCORES = 8
BPC = B // NCORES   # 8 batches per core
P = 128             # SBUF partitions
KC = H // P         # 4 k-chunks per batch
NBLK = BPC * KC     # 32 moving blocks per core
MMF = 512           # matmul moving free width (one PSUM bank of fp32)
NK = 16             # patched candidates per row (2 rounds of max8)
PW = 516            # gather row width: enc16(512) + emb16(3) + pad
F32 = mybir.dt.float32
F16 = mybir.dt.float16
F8E3 = mybir.dt.float8e3
I32 = mybir.dt.int32
U32 = mybir.dt.uint32

DEFAULT_VARIANT = os.environ.get("ATTN_VARIANT", "v6q4")

# Last BassKernelResults (for test harness to read exec_time_ns)
LAST_RESULTS = None


@with_exitstack
def emit_kernel(ctx, tc, out_ap, x_ap, stat_ap, aff_ap, reps=1):
    """fp16 "full" variant: stationary one-hot-column h blocks, fp16 enc
    stream, host-precomputed aff bias, softmax epilogue."""
    nc = tc.nc
    xv = x_ap.rearrange("(n p) s -> n p s", p=P)          # [32, 128, 2048]
    statv = stat_ap.rearrange("p (n j) -> p n j", j=BPC)  # [128, 32, 8]

    singles = ctx.enter_context(tc.tile_pool(name="singles", bufs=1))
    bpool = ctx.enter_context(tc.tile_pool(name="blocks", bufs=8))
    smpool = ctx.enter_context(tc.tile_pool(name="smx", bufs=2))
    epool = ctx.enter_context(tc.tile_pool(name="es", bufs=4))
    ppool = ctx.enter_context(tc.tile_pool(name="psums", bufs=2, space="PSUM"))

    statt = singles.tile([P, NBLK, BPC], F16)
    nc.gpsimd.dma_start(out=statt[:, :, :], in_=statv)
    afft = singles.tile([BPC, S], F32)
    nc.gpsimd.dma_start(out=afft[:, :], in_=aff_ap)

    nmm = S // MMF
    for _ in range(reps):
        energy = ppool.tile([BPC, S], F32)
        for i in range(NBLK):
            blk = bpool.tile([P, S], F16)
            q = nc.sync if i % 2 == 0 else nc.scalar
            q.dma_start(out=blk[:, :], in_=xv[i])
            first = i == 0
            last = i == NBLK - 1
            for sc in range(nmm):
                nc.tensor.matmul(
                    energy[:, sc * MMF : (sc + 1) * MMF],
                    statt[:, i, :],
                    blk[:, sc * MMF : (sc + 1) * MMF],
                    start=first,
                    stop=last,
                )

        eng = smpool.tile([BPC, S], F32)
        nc.vector.tensor_tensor(
            eng[:, :], energy[:, :], afft[:, :], mybir.AluOpType.add
        )
        negmax1 = epool.tile([BPC, 1], F32)
        nc.vector.reduce_max(
            negmax1[:, :], eng[:, : S // 2], axis=mybir.AxisListType.X,
            negate=True,
        )
        negmax = epool.tile([BPC, 1], F32)
        nc.vector.reduce_max(
            negmax[:, :], eng[:, S // 2 :], axis=mybir.AxisListType.X,
            negate=True,
        )
        nc.vector.tensor_tensor(
            negmax[:, :], negmax[:, :], negmax1[:, :], mybir.AluOpType.min
        )
        expT = smpool.tile([BPC, S], F32)
        sums = epool.tile([BPC, 1], F32)
        nc.scalar.activation(
            expT[:, :],
            eng[:, :],
            mybir.ActivationFunctionType.Exp,
            bias=negmax[:, :],
            scale=1.0,
            accum_out=sums[:, :],
        )
        rsum = epool.tile([BPC, 1], F32)
        nc.vector.reciprocal(rsum[:, :], sums[:, :])
        outT = smpool.tile([BPC, S], F32)
        nc.scalar.activation(
            outT[:, :],
            expT[:, :],
            mybir.ActivationFunctionType.Copy,
            bias=0.0,
            scale=rsum[:, :],
        )
        nc.sync.dma_start(out=out_ap, in_=outT[:, :])


@with_exitstack
def emit_kernel_v3(ctx, tc, out_ap, x_ap, stat_ap, embt_ap, vstat_ap, p2_ap,
                   hsel_ap, bc8_ap, bc2048_ap, reps=1, stage="all",
                   pipelined=True):
    """fp8e3 two-pass variant. stage: 'p1' = pass-1 only (timing floor),
    'nog' = extraction but no gather/patch (plain softmax of fp8 energies
    with top-16 zeroed - wrong output, DVE-chain timing), 'nos' = gather
    but no scatter, 'all' = full."""
    nc = tc.nc
    xv = x_ap.rearrange("(n p) s -> n p s", p=P)          # [32, 128, 2048] f8
    statv = stat_ap.rearrange("p (n j) -> p n j", j=BPC)  # [128, 32, 8] f8
    outv = out_ap.rearrange("(b s) o -> b (s o)", b=BPC)  # [8, 2048]

    singles = ctx.enter_context(tc.tile_pool(name="singles", bufs=1))
    bpool = ctx.enter_context(tc.tile_pool(name="blocks", bufs=8))
    empool = ctx.enter_context(tc.tile_pool(name="embts", bufs=2))
    spool = ctx.enter_context(tc.tile_pool(name="sm", bufs=2))
    tpool = ctx.enter_context(tc.tile_pool(name="tiny", bufs=3))
    gpool = ctx.enter_context(tc.tile_pool(name="gath", bufs=2))
    ppool = ctx.enter_context(tc.tile_pool(name="psums", bufs=2, space="PSUM"))

    # one-time loads on the gpsimd (SWDGE) queue
    statt = singles.tile([P, NBLK, BPC], F8E3)
    nc.gpsimd.dma_start(out=statt[:, :, :], in_=statv)
    vstatt = singles.tile([BPC * A, BPC], F16)
    nc.gpsimd.dma_start(out=vstatt[:, :], in_=vstat_ap)
    hselt = singles.tile([P, PW], F16)
    nc.gpsimd.dma_start(out=hselt[:, :], in_=hsel_ap)
    bc8t = singles.tile([P, 1], F32)
    nc.gpsimd.dma_start(out=bc8t[:, :], in_=bc8_ap)
    bc2048t = singles.tile([P, 1], F32)
    nc.gpsimd.dma_start(out=bc2048t[:, :], in_=bc2048_ap)

    nmm = S // MMF

    if stage == "dma":
        outT0 = singles.tile([BPC, S], F32)
        nc.vector.memset(outT0[:, :], 0.0)
        for _ in range(reps):
            for i in range(NBLK):
                blk = bpool.tile([P, S], F8E3)
                q = nc.sync if i % 2 == 0 else nc.scalar
                q.dma_start(out=blk[:, :], in_=xv[i])
                ec = tpool.tile([P, 1], F8E3)
                nc.vector.tensor_copy(ec[:, :], blk[:, 0:1])
            embt = empool.tile([BPC * A, S], F16)
            nc.scalar.dma_start(out=embt[:, :], in_=embt_ap)
            ec2 = tpool.tile([BPC * A, 1], F16)
            nc.vector.tensor_copy(ec2[:, :], embt[:, 0:1])
        nc.gpsimd.dma_start(out=outv, in_=outT0[:, :])
        return

    def pass1():
        # ---- pass 1: energy [8, 2048] accumulated in PSUM ----
        energy = ppool.tile([BPC, S], F32)
        for i in range(NBLK):
            blk = bpool.tile([P, S], F8E3)
            q = nc.sync if i % 2 == 0 else nc.scalar
            q.dma_start(out=blk[:, :], in_=xv[i])
            for sc in range(nmm):
                nc.tensor.matmul(
                    energy[:, sc * MMF : (sc + 1) * MMF],
                    statt[:, i, :],
                    blk[:, sc * MMF : (sc + 1) * MMF],
                    start=(i == 0),
                    stop=False,
                )
        # affect term: 3 fp16 contraction rows per batch
        embt = empool.tile([BPC * A, S], F16)
        nc.scalar.dma_start(out=embt[:, :], in_=embt_ap)
        for sc in range(nmm):
            nc.tensor.matmul(
                energy[:, sc * MMF : (sc + 1) * MMF],
                vstatt[:, :],
                embt[:, sc * MMF : (sc + 1) * MMF],
                start=False,
                stop=True,
            )
        return energy

    def stageA(energy):
        """Extraction + gather launch. The gather's latency is absorbed by
        running stageB one iteration later."""
        eng = spool.tile([BPC, S], F32)
        nc.vector.tensor_copy(eng[:, :], energy[:, :])  # frees PSUM buf

        if stage == "p1":
            e0 = tpool.tile([BPC, 1], F32)
            nc.vector.tensor_copy(e0[:, :], eng[:, 0:1])
            nc.sync.dma_start(out=outv[:, 0:1], in_=e0[:, :])
            return None

        # top-16 per row: values+indices, masked out of the tail in place
        m1 = tpool.tile([BPC, 8], F32)
        nc.vector.max(m1[:, :], eng[:, :])
        iall = tpool.tile([BPC, NK], U32)
        nc.vector.max_index(iall[:, 0:8], m1[:, :], eng[:, :])
        eng2 = spool.tile([BPC, S], F32)
        nc.vector.match_replace(eng2[:, :], m1[:, :], eng[:, :], -1e30)
        m2 = tpool.tile([BPC, 8], F32)
        nc.vector.max(m2[:, :], eng2[:, :])
        nc.vector.max_index(iall[:, 8:16], m2[:, :], eng2[:, :])
        eng3 = spool.tile([BPC, S], F32)
        nc.vector.match_replace(eng3[:, :], m2[:, :], eng2[:, :], -1e30)

        negmax = tpool.tile([BPC, 1], F32)
        nc.vector.tensor_scalar_mul(negmax[:, :], m1[:, 0:1], -1.0)
        if32 = tpool.tile([BPC, NK], F32)
        nc.vector.tensor_copy(if32[:, :], iall[:, :])
        # rearrange candidates [8, 16] -> [128, 1] (partition-major)
        cidx = gpool.tile([P, 1], F32)
        nc.gpsimd.dma_start(out=cidx[:, :], in_=if32[:, :])

        G = None
        if stage not in ("nog",):
            # gather exact fp16 rows: p2 row = s*8 + b
            crowf = gpool.tile([P, 1], F32)
            nc.vector.tensor_scalar_mul(crowf[:, :], cidx[:, :], 8.0)
            nc.vector.tensor_tensor(
                crowf[:, :], crowf[:, :], bc8t[:, :], mybir.AluOpType.add
            )
            crow = gpool.tile([P, 1], I32)
            nc.vector.tensor_copy(crow[:, :], crowf[:, :])
            G = gpool.tile([P, PW], F16)
            nc.gpsimd.indirect_dma_start(
                out=G[:, :],
                out_offset=None,
                in_=p2_ap,
                in_offset=IndirectOffsetOnAxis(ap=crow[:, 0:1], axis=0),
            )
        return (eng3, negmax, cidx, G)

    def stageB(st):
        eng3, negmax, cidx, G = st
        if stage not in ("nog",):
            # exact energy per candidate
            prod = gpool.tile([P, PW], F32)
            nc.vector.tensor_tensor(
                prod[:, :], G[:, :], hselt[:, :], mybir.AluOpType.mult
            )
            ee = gpool.tile([P, 1], F32)
            nc.vector.reduce_sum(
                ee[:, 0:1], prod[:, :], axis=mybir.AxisListType.X
            )
        if stage not in ("nog", "gonly"):
            # back to [8, 16] layout for per-row reduction
            eeb = tpool.tile([BPC, NK], F32)
            nc.gpsimd.dma_start(out=eeb[:, :], in_=ee[:, :])
            expfix = tpool.tile([BPC, NK], F32)
            nc.scalar.activation(
                expfix[:, :], eeb[:, :],
                mybir.ActivationFunctionType.Exp,
                bias=negmax[:, :], scale=1.0,
            )
            psumf = tpool.tile([BPC, 1], F32)
            nc.vector.reduce_sum(
                psumf[:, 0:1], expfix[:, :], axis=mybir.AxisListType.X
            )

        # tail softmax (top-16 already -1e30 in eng3)
        exps = spool.tile([BPC, S], F32)
        tsum = tpool.tile([BPC, 1], F32)
        nc.scalar.activation(
            exps[:, :],
            eng3[:, :],
            mybir.ActivationFunctionType.Exp,
            bias=negmax[:, :],
            scale=1.0,
            accum_out=tsum[:, :],
        )
        zt = tpool.tile([BPC, 1], F32)
        if stage == "gonly":
            # consume ee so the gather isn't dead code; ee*0 keeps values
            nc.vector.scalar_tensor_tensor(
                out=zt[:, :], in0=ee[0:BPC, 0:1], scalar=0.0,
                in1=tsum[:, :],
                op0=mybir.AluOpType.mult, op1=mybir.AluOpType.add,
            )
        elif stage != "nog":
            nc.vector.tensor_tensor(
                zt[:, :], tsum[:, :], psumf[:, :], mybir.AluOpType.add
            )
        else:
            nc.vector.tensor_copy(zt[:, :], tsum[:, :])
        rsum = tpool.tile([BPC, 1], F32)
        nc.vector.reciprocal(rsum[:, :], zt[:, :])
        outT = spool.tile([BPC, S], F32)
        nc.vector.tensor_scalar_mul(outT[:, :], exps[:, :], rsum[:, 0:1])

        # base write then sparse patches, both on gpsimd queue (ordered)
        nc.gpsimd.dma_start(out=outv, in_=outT[:, :])
        if stage == "all":
            pv8 = tpool.tile([BPC, NK], F32)
            nc.vector.tensor_scalar_mul(pv8[:, :], expfix[:, :], rsum[:, 0:1])
            pv = gpool.tile([P, 1], F32)
            nc.gpsimd.dma_start(out=pv[:, :], in_=pv8[:, :])
            offf = gpool.tile([P, 1], F32)
            nc.vector.tensor_tensor(
                offf[:, :], cidx[:, :], bc2048t[:, :], mybir.AluOpType.add
            )
            offi = gpool.tile([P, 1], I32)
            nc.vector.tensor_copy(offi[:, :], offf[:, :])
            nc.gpsimd.indirect_dma_start(
                out=out_ap,
                out_offset=IndirectOffsetOnAxis(ap=offi[:, 0:1], axis=0),
                in_=pv[:, :],
                in_offset=None,
            )

    # two-stage software pipeline: stageB(r-2) then stageA(r-1) are emitted
    # after pass1(r), so (a) every queue sees the next iteration's DMA
    # issues before the late-ready epilogue compute, and (b) the indirect
    # gather launched in stageA(r) has a full iteration of latency budget
    # before stageB(r) consumes it - without this the base-output write
    # (which needs Z = tail + patch sum) serializes on the gather roundtrip.
    if pipelined:
        prevE = None
        prevS = None
        for _ in range(reps):
            cur = pass1()
            if prevS is not None:
                stageB(prevS)
                prevS = None
            if prevE is not None:
                prevS = stageA(prevE)
            prevE = cur
        if prevS is not None:
            stageB(prevS)
        if prevE is not None:
            st = stageA(prevE)
            if st is not None:
                stageB(st)
    else:
        for _ in range(reps):
            st = stageA(pass1())
            if st is not None:
                stageB(st)


NK6 = 8              # v6: one max round
NC6 = BPC * NK6      # 64 gather candidates
NBLK6 = NBLK // 2    # 16 paired moving blocks
F8E4 = mybir.dt.float8e4


@with_exitstack
def emit_kernel_v6(ctx, tc, out_ap, x_ap, stat_ap, embt_ap, vstat_ap, p2_ap,
                   hsel_ap, bc8_ap, bc2048_ap, reps=1, stage="all", q3=False):
    """DoubleRow fp8e4 pass-1 (0.5 cyc/row) + top-8 exact-patch pass-2.
    Depth-2 software pipeline as in v3."""
    nc = tc.nc
    GQ4 = (1, 4, 7, 10, 13)
    # block j = b*2 + kp: [128, 2, 2048]; row p, ktile t = enc[s, b, (2kp+t)*128+p]
    xv = x_ap.rearrange("(n p) (t s) -> n p t s", p=P, t=2)   # [16,128,2,2048]
    # stationary free dim padded to 16 (DoubleRow ISA requires M >= 16)
    statv = stat_ap.rearrange("p (n t j) -> p n t j", t=2, j=2 * BPC)
    outv = out_ap.rearrange("(b s) o -> b (s o)", b=BPC)      # [8, 2048]

    singles = ctx.enter_context(tc.tile_pool(name="singles", bufs=1))
    bpool = ctx.enter_context(tc.tile_pool(name="blocks", bufs=6))
    empool = ctx.enter_context(tc.tile_pool(name="embts", bufs=2))
    spool = ctx.enter_context(tc.tile_pool(name="sm", bufs=2))
    tpool = ctx.enter_context(tc.tile_pool(name="tiny", bufs=3))
    gpool = ctx.enter_context(tc.tile_pool(name="gath", bufs=2))
    ppool = ctx.enter_context(tc.tile_pool(name="psums", bufs=2, space="PSUM"))

    statt = singles.tile([P, NBLK6, 2, 2 * BPC], F8E4)
    nc.gpsimd.dma_start(out=statt[:, :, :, :], in_=statv)
    vstatt = singles.tile([BPC * A, 2 * BPC], F16)
    nc.gpsimd.dma_start(out=vstatt[:, :], in_=vstat_ap)
    hselt = singles.tile([NC6, PW], F16)
    nc.gpsimd.dma_start(out=hselt[:, :], in_=hsel_ap)
    bc8t = singles.tile([NC6, 1], F32)
    nc.gpsimd.dma_start(out=bc8t[:, :], in_=bc8_ap)
    bc2048t = singles.tile([NC6, 1], F32)
    nc.gpsimd.dma_start(out=bc2048t[:, :], in_=bc2048_ap)

    nmm = S // MMF

    def pass1():
        energy = ppool.tile([2 * BPC, S], F32)
        for j in range(NBLK6):
            blk = bpool.tile([P, 2, S], F8E4)
            if q3 and j in (GQ4 if q3 == 2 else (2, 7, 12)):
                q = nc.gpsimd
            elif j % 2 == 0:
                q = nc.sync
            else:
                q = nc.scalar
            q.dma_start(out=blk[:, :, :], in_=xv[j])
            for sc in range(nmm):
                nc.tensor.matmul(
                    energy[:, sc * MMF : (sc + 1) * MMF],
                    statt[:, j, :, :],
                    blk[:, :, sc * MMF : (sc + 1) * MMF],
                    start=(j == 0),
                    stop=False,
                    perf_mode=mybir.MatmulPerfMode.DoubleRow,
                )
        embt = empool.tile([BPC * A, S], F16)
        nc.scalar.dma_start(out=embt[:, :], in_=embt_ap)
        for sc in range(nmm):
            nc.tensor.matmul(
                energy[:, sc * MMF : (sc + 1) * MMF],
                vstatt[:, :],
                embt[:, sc * MMF : (sc + 1) * MMF],
                start=False,
                stop=True,
            )
        return energy

    def stageA(energy):
        eng = spool.tile([BPC, S], F32)
        nc.vector.tensor_copy(eng[:, :], energy[0:BPC, :])  # frees PSUM buf

        if stage == "p1":
            e0 = tpool.tile([BPC, 1], F32)
            nc.vector.tensor_copy(e0[:, :], eng[:, 0:1])
            nc.sync.dma_start(out=outv[:, 0:1], in_=e0[:, :])
            return None

        m1 = tpool.tile([BPC, 8], F32)
        nc.vector.max(m1[:, :], eng[:, :])
        i8 = tpool.tile([BPC, NK6], U32)
        nc.vector.max_index(i8[:, :], m1[:, :], eng[:, :])
        eng3 = spool.tile([BPC, S], F32)
        nc.vector.match_replace(eng3[:, :], m1[:, :], eng[:, :], -1e30)

        negmax = tpool.tile([BPC, 1], F32)
        nc.vector.tensor_scalar_mul(negmax[:, :], m1[:, 0:1], -1.0)
        if32 = tpool.tile([BPC, NK6], F32)
        nc.vector.tensor_copy(if32[:, :], i8[:, :])
        cidx = gpool.tile([NC6, 1], F32)
        nc.gpsimd.dma_start(out=cidx[:, :], in_=if32[:, :])

        crowf = gpool.tile([NC6, 1], F32)
        nc.vector.tensor_scalar(
            out=crowf[:, :], in0=cidx[:, :], scalar1=8.0, scalar2=bc8t[:, 0:1],
            op0=mybir.AluOpType.mult, op1=mybir.AluOpType.add,
        )
        crow = gpool.tile([NC6, 1], I32)
        nc.vector.tensor_copy(crow[:, :], crowf[:, :])
        G = gpool.tile([NC6, PW], F16)
        nc.gpsimd.indirect_dma_start(
            out=G[:, :],
            out_offset=None,
            in_=p2_ap,
            in_offset=IndirectOffsetOnAxis(ap=crow[:, 0:1], axis=0),
        )
        return (eng3, negmax, cidx, G)

    def stageB(st):
        eng3, negmax, cidx, G = st
        prod = gpool.tile([NC6, PW], F32)
        nc.vector.tensor_tensor(
            prod[:, :], G[:, :], hselt[:, :], mybir.AluOpType.mult
        )
        ee = gpool.tile([NC6, 1], F32)
        nc.vector.reduce_sum(ee[:, 0:1], prod[:, :], axis=mybir.AxisListType.X)
        eeb = tpool.tile([BPC, NK6], F32)
        nc.gpsimd.dma_start(out=eeb[:, :], in_=ee[:, :])
        expfix = tpool.tile([BPC, NK6], F32)
        nc.scalar.activation(
            expfix[:, :], eeb[:, :],
            mybir.ActivationFunctionType.Exp,
            bias=negmax[:, :], scale=1.0,
        )
        psumf = tpool.tile([BPC, 1], F32)
        nc.vector.reduce_sum(
            psumf[:, 0:1], expfix[:, :], axis=mybir.AxisListType.X
        )

        exps = spool.tile([BPC, S], F32)
        tsum = tpool.tile([BPC, 1], F32)
        nc.scalar.activation(
            exps[:, :],
            eng3[:, :],
            mybir.ActivationFunctionType.Exp,
            bias=negmax[:, :],
            scale=1.0,
            accum_out=tsum[:, :],
        )
        zt = tpool.tile([BPC, 1], F32)
        nc.vector.tensor_tensor(
            zt[:, :], tsum[:, :], psumf[:, :], mybir.AluOpType.add
        )
        rsum = tpool.tile([BPC, 1], F32)
        nc.vector.reciprocal(rsum[:, :], zt[:, :])
        outT = spool.tile([BPC, S], F32)
        nc.vector.tensor_scalar_mul(outT[:, :], exps[:, :], rsum[:, 0:1])

        nc.gpsimd.dma_start(out=outv, in_=outT[:, :])
        pv8 = tpool.tile([BPC, NK6], F32)
        nc.vector.tensor_scalar_mul(pv8[:, :], expfix[:, :], rsum[:, 0:1])
        pv = gpool.tile([NC6, 1], F32)
        nc.gpsimd.dma_start(out=pv[:, :], in_=pv8[:, :])
        offf = gpool.tile([NC6, 1], F32)
        nc.vector.tensor_scalar_add(offf[:, :], cidx[:, :], bc2048t[:, 0:1])
        offi = gpool.tile([NC6, 1], I32)
        nc.vector.tensor_copy(offi[:, :], offf[:, :])
        nc.gpsimd.indirect_dma_start(
            out=out_ap,
            out_offset=IndirectOffsetOnAxis(ap=offi[:, 0:1], axis=0),
            in_=pv[:, :],
            in_offset=None,
        )

    prevE = None
    prevS = None
    for _ in range(reps):
        cur = pass1()
        if prevS is not None:
            stageB(prevS)
            prevS = None
        if prevE is not None:
            prevS = stageA(prevE)
        prevE = cur
    if prevS is not None:
        stageB(prevS)
    if prevE is not None:
        st = stageA(prevE)
        if st is not None:
            stageB(st)


def make_in_maps_v6(hidden, encoder_outputs, embedding, affect_matrix):
    import ml_dtypes

    f8 = np.dtype(ml_dtypes.float8_e4m3)
    hidden = np.asarray(hidden, dtype=np.float32)
    enc = np.asarray(encoder_outputs, dtype=np.float32)
    emb = np.asarray(embedding, dtype=np.float32)
    am = np.asarray(affect_matrix, dtype=np.float32)

    h = hidden[0]
    v32 = h @ am
    h8 = h.astype(f8)
    enc8 = enc.astype(f8)
    h16 = h.astype(np.float16)
    v16 = v32.astype(np.float16)
    enc16 = enc.astype(np.float16)
    emb16 = emb.astype(np.float16)

    cc = np.arange(NC6)
    bc8 = (cc // NK6).astype(np.float32)[:, None]
    bc2048 = (cc // NK6 * S).astype(np.float32)[:, None]

    in_maps = []
    for c in range(NCORES):
        lo, hi = c * BPC, (c + 1) * BPC
        # [S, 8, 512] -> [8b, 2kp, 2t, 128p, 2048s] -> rows (b,kp,p), cols (t,s)
        ec = np.transpose(enc8[:, lo:hi, :], (1, 2, 0)).reshape(
            BPC, KC // 2, 2, P, S
        )
        xp = np.ascontiguousarray(
            np.transpose(ec, (0, 1, 3, 2, 4))
        ).reshape(NBLK6 * P, 2 * S)
        hh8 = h8[lo:hi].reshape(BPC, KC // 2, 2, P)
        stat = np.zeros((P, NBLK6, 2, 2 * BPC), f8)
        for b in range(BPC):
            for kp in range(KC // 2):
                for t in range(2):
                    stat[:, b * (KC // 2) + kp, t, b] = hh8[b, kp, t]
        embt = np.ascontiguousarray(
            np.transpose(emb16[:, lo:hi, :], (1, 2, 0))
        ).reshape(BPC * A, S)
        vstat = np.zeros((BPC * A, 2 * BPC), np.float16)
        for b in range(BPC):
            vstat[b * A : (b + 1) * A, b] = v16[lo + b]
        p2 = np.zeros((S * BPC, PW), np.float16)
        p2[:, :H] = enc16[:, lo:hi, :].reshape(S * BPC, H)
        p2[:, H : H + A] = emb16[:, lo:hi, :].reshape(S * BPC, A)
        hsel = np.zeros((NC6, PW), np.float16)
        hsel[:, :H] = h16[lo + cc // NK6]
        hsel[:, H : H + A] = v16[lo + cc // NK6]
        in_maps.append(
            {
                "x": xp,
                "stat": stat.reshape(P, NBLK6 * 2 * 2 * BPC),
                "embt": embt,
                "vstat": vstat,
                "p2": p2,
                "hsel": hsel,
                "bc8": bc8,
                "bc2048": bc2048,
            }
        )
    return in_maps


_NC_CACHE = {}


def build_nc(reps=1, variant="v3"):
    key = (reps, variant)
    if key in _NC_CACHE:
        return _NC_CACHE[key]
    nc = bacc.Bacc(
        "TRN2",
        target_bir_lowering=False,
        debug=False,
        enable_asserts=False,
        num_devices=NCORES,
    )
    if variant.startswith("v6"):
        x = nc.dram_tensor(
            "x", [NBLK6 * P, 2 * S], F8E4, kind="ExternalInput"
        ).ap()
        stat = nc.dram_tensor(
            "stat", [P, NBLK6 * 2 * 2 * BPC], F8E4, kind="ExternalInput"
        ).ap()
        embt = nc.dram_tensor(
            "embt", [BPC * A, S], F16, kind="ExternalInput"
        ).ap()
        vstat = nc.dram_tensor(
            "vstat", [BPC * A, 2 * BPC], F16, kind="ExternalInput"
        ).ap()
        p2 = nc.dram_tensor(
            "p2", [S * BPC, PW], F16, kind="ExternalInput"
        ).ap()
        hsel = nc.dram_tensor(
            "hsel", [NC6, PW], F16, kind="ExternalInput"
        ).ap()
        bc8 = nc.dram_tensor("bc8", [NC6, 1], F32, kind="ExternalInput").ap()
        bc2048 = nc.dram_tensor(
            "bc2048", [NC6, 1], F32, kind="ExternalInput"
        ).ap()
        out = nc.dram_tensor(
            "out", [BPC * S, 1], F32, kind="ExternalOutput"
        ).ap()
        stage = {"v6p1": "p1", "v6q3p1": "p1"}.get(variant, "all")
        with tile.TileContext(nc) as tc:
            emit_kernel_v6(
                tc, out, x, stat, embt, vstat, p2, hsel, bc8, bc2048,
                reps=reps, stage=stage,
                q3=(2 if "q4" in variant else (1 if "q3" in variant else 0)),
            )
    elif variant.startswith("v3"):
        x = nc.dram_tensor("x", [NBLK * P, S], F8E3, kind="ExternalInput").ap()
        stat = nc.dram_tensor(
            "stat", [P, NBLK * BPC], F8E3, kind="ExternalInput"
        ).ap()
        embt = nc.dram_tensor(
            "embt", [BPC * A, S], F16, kind="ExternalInput"
        ).ap()
        vstat = nc.dram_tensor(
            "vstat", [BPC * A, BPC], F16, kind="ExternalInput"
        ).ap()
        p2 = nc.dram_tensor(
            "p2", [S * BPC, PW], F16, kind="ExternalInput"
        ).ap()
        hsel = nc.dram_tensor("hsel", [P, PW], F16, kind="ExternalInput").ap()
        bc8 = nc.dram_tensor("bc8", [P, 1], F32, kind="ExternalInput").ap()
        bc2048 = nc.dram_tensor(
            "bc2048", [P, 1], F32, kind="ExternalInput"
        ).ap()
        out = nc.dram_tensor(
            "out", [BPC * S, 1], F32, kind="ExternalOutput"
        ).ap()
        stage = {"v3p1": "p1", "v3nog": "nog", "v3nos": "nos",
                 "v3gonly": "gonly", "v3dma": "dma",
                 "v3s": "all"}.get(variant, "all")
        with tile.TileContext(nc) as tc:
            emit_kernel_v3(
                tc, out, x, stat, embt, vstat, p2, hsel, bc8, bc2048,
                reps=reps, stage=stage, pipelined=(variant != "v3s"),
            )
    else:
        x = nc.dram_tensor("x", [NBLK * P, S], F16, kind="ExternalInput").ap()
        stat = nc.dram_tensor(
            "stat", [P, NBLK * BPC], F16, kind="ExternalInput"
        ).ap()
        aff = nc.dram_tensor("aff", [BPC, S], F32, kind="ExternalInput").ap()
        out = nc.dram_tensor("out", [BPC, S], F32, kind="ExternalOutput").ap()
        with tile.TileContext(nc) as tc:
            emit_kernel(tc, out, x, stat, aff, reps=reps)
    nc.compile()
    _NC_CACHE[key] = nc
    return nc


def make_in_maps(hidden, encoder_outputs, embedding, affect_matrix):
    hidden = np.asarray(hidden, dtype=np.float32)
    enc = np.asarray(encoder_outputs, dtype=np.float32)
    emb = np.asarray(embedding, dtype=np.float32)
    am = np.asarray(affect_matrix, dtype=np.float32)

    h = hidden[0]                                   # [B, H]
    v = h @ am                                      # [B, A]
    aff = np.einsum("ba,sba->sb", v, emb).astype(np.float32)  # [S, B]
    h16 = h.astype(np.float16)
    enc16 = enc.astype(np.float16)                  # [S, B, H]

    in_maps = []
    for c in range(NCORES):
        lo, hi = c * BPC, (c + 1) * BPC
        xp = np.ascontiguousarray(
            np.transpose(enc16[:, lo:hi, :], (1, 2, 0))
        ).reshape(BPC * H, S)
        hh = h16[lo:hi].reshape(BPC, KC, P)
        stat = np.zeros((P, NBLK, BPC), np.float16)
        for b in range(BPC):
            for kc in range(KC):
                stat[:, b * KC + kc, b] = hh[b, kc]
        in_maps.append(
            {
                "x": xp,
                "stat": stat.reshape(P, NBLK * BPC),
                "aff": np.ascontiguousarray(aff[:, lo:hi].T),
            }
        )
    return in_maps


def make_in_maps_v3(hidden, encoder_outputs, embedding, affect_matrix):
    import ml_dtypes

    f8 = np.dtype(ml_dtypes.float8_e3m4)
    hidden = np.asarray(hidden, dtype=np.float32)
    enc = np.asarray(encoder_outputs, dtype=np.float32)
    emb = np.asarray(embedding, dtype=np.float32)
    am = np.asarray(affect_matrix, dtype=np.float32)

    h = hidden[0]                                   # [B, H]
    v32 = h @ am                                    # [B, A]
    h8 = h.astype(f8)
    enc8 = enc.astype(f8)
    h16 = h.astype(np.float16)
    v16 = v32.astype(np.float16)
    enc16 = enc.astype(np.float16)
    emb16 = emb.astype(np.float16)

    cc = np.arange(P)
    bc8 = (cc // NK).astype(np.float32)[:, None]
    bc2048 = (cc // NK * S).astype(np.float32)[:, None]

    in_maps = []
    for c in range(NCORES):
        lo, hi = c * BPC, (c + 1) * BPC
        xp = np.ascontiguousarray(
            np.transpose(enc8[:, lo:hi, :], (1, 2, 0))
        ).reshape(NBLK * P, S)
        hh8 = h8[lo:hi].reshape(BPC, KC, P)
        stat = np.zeros((P, NBLK, BPC), f8)
        for b in range(BPC):
            for kc in range(KC):
                stat[:, b * KC + kc, b] = hh8[b, kc]
        # affect rows: embt[b*A+a, s] = emb[s, lo+b, a]
        embt = np.ascontiguousarray(
            np.transpose(emb16[:, lo:hi, :], (1, 2, 0))
        ).reshape(BPC * A, S)
        vstat = np.zeros((BPC * A, BPC), np.float16)
        for b in range(BPC):
            vstat[b * A : (b + 1) * A, b] = v16[lo + b]
        # gather table rows r = s*8 + b: [enc16(512), emb16(3), pad]
        p2 = np.zeros((S * BPC, PW), np.float16)
        p2[:, :H] = enc16[:, lo:hi, :].reshape(S * BPC, H)
        p2[:, H : H + A] = emb16[:, lo:hi, :].reshape(S * BPC, A)
        hsel = np.zeros((P, PW), np.float16)
        hsel[:, :H] = h16[lo + cc // NK]
        hsel[:, H : H + A] = v16[lo + cc // NK]
        in_maps.append(
            {
                "x": xp,
                "stat": stat.reshape(P, NBLK * BPC),
                "embt": embt,
                "vstat": vstat,
                "p2": p2,
                "hsel": hsel,
                "bc8": bc8,
                "bc2048": bc2048,
            }
        )
    return in_maps


def kernel(hidden, encoder_outputs, embedding, affect_matrix):
    global LAST_RESULTS
    variant = DEFAULT_VARIANT
    nc = build_nc(variant=variant)
    if variant.startswith("v6"):
        in_maps = make_in_maps_v6(
            hidden, encoder_outputs, embedding, affect_matrix
        )
    elif variant.startswith("v3"):
        in_maps = make_in_maps_v3(
            hidden, encoder_outputs, embedding, affect_matrix
        )
    else:
        in_maps = make_in_maps(
            hidden, encoder_outputs, embedding, affect_matrix
        )
    last_exc = None
    for attempt in range(3):
        try:
            res = run_bass_kernel_spmd(
                nc,
                in_maps,
                core_ids=list(range(NCORES)),
                trace=bool(int(os.environ.get("ATTN_TRACE", "0"))),
            )
            break
        except Exception as e:  # transient wedged-device errors recover on retry
            last_exc = e
            if attempt == 2:
                raise
            import time as _time

            _time.sleep(5.0)
    LAST_RESULTS = res
    outs = [r["out"].reshape(BPC, S) for r in res.results]
    full = np.concatenate(outs, axis=0)             # [B, S]
    return full[:, None, :].astype(np.float32)      # [B, 1, S]
